# revision 1
# baseline (speedup 1.0000x reference)
"""Paged KV-cache gather + dequant kernel for 8 Trainium2 NeuronCores.

Problem: out[0] = zeros; out[1+i] = kv_cache[block_tables.flat[i]] * scale
(k_scale for the K half, v_scale for the V half), zeroed where the table
entry is <= 0.  Shapes: kv_cache [4096, 2, 8, 16, 128] fp16,
block_tables [32, 128] int, out [4097, 2, 8, 16, 128] fp16.

Sharding: batch across the 8 cores (4 sequences = 512 entries per core);
kv_cache replicated.  Per core the kernel views kv_cache as 8192 rows of
16384 fp16 (one row = one K or V half-block, 32 KB) and:
  1. loads block_tables, builds int16 row indices (2*bt for K, 2*bt+1 for V)
     wrapped in 16 partitions and replicated across the 8 GPSIMD cores,
  2. dma_gather's 128 rows per call into [128, 16384] SBUF tiles,
  3. multiplies by a per-partition scalar (valid * k/v_scale) on DVE,
  4. stores rows to the output shard with a strided HWDGE DMA.
Invalid entries gather row 0/1 and are zeroed by the scale; output block 0
is never written (ExternalOutput buffers are zero-initialized).
"""

import sys

if "/opt/trn_rl_repo" not in sys.path:
    sys.path.insert(0, "/opt/trn_rl_repo")

from contextlib import ExitStack

import numpy as np

import concourse.bacc as bacc
import concourse.bass as bass
import concourse.mybir as mybir
from concourse import bass_utils
from concourse._compat import get_trn_type
from concourse.library_config import mlp

N_CORES = 8
NUM_BLOCKS, NUM_KV_HEADS, HEAD_DIM, BLOCK_SIZE = 4096, 8, 128, 16
BATCH, MAX_BLOCKS_PER_SEQ = 32, 128

ROW = NUM_KV_HEADS * BLOCK_SIZE * HEAD_DIM  # 16384 fp16 = one K or V half-block
N_ROWS = NUM_BLOCKS * 2                     # 8192
E_PER_CORE = (BATCH // N_CORES) * MAX_BLOCKS_PER_SEQ  # 512 entries per core
N_CHUNK = E_PER_CORE // 128                 # 4 gather chunks per K/V half
N_BUF = 3                                   # SBUF pipeline depth

_NC_CACHE = None


def build_nc(
    n_reps: int = 1,
    chunk: int = 128,
    n_buf: int = 3,
    single_packet: bool = True,
    n_queues: int = 1,
    dual_store: bool = False,
) -> bass.Bass:
    # Bacc (not raw Bass): dma_gather's register operand needs the bacc
    # reg-alloc/lowering pass or walrus rejects the ISA encoding.
    # n_reps > 1 unrolls the main loop for benchmarking (same output).
    nc = bacc.Bacc(get_trn_type() or "TRN2", num_swdge_queues=n_queues)

    slots = chunk // 128        # buffer slots per gather
    g_per_half = E_PER_CORE // chunk  # gathers per K/V half
    n_gather = 2 * g_per_half   # gathers per rep
    cols = chunk // 16          # idx columns per gather

    kv = nc.dram_tensor("kv", [N_ROWS, ROW], mybir.dt.float16, kind="ExternalInput")
    bt = nc.dram_tensor("bt", [E_PER_CORE], mybir.dt.int32, kind="ExternalInput")
    scales = nc.dram_tensor("scales", [128, 2], mybir.dt.float32, kind="ExternalInput")
    # out block 0 stays zero (buffers are pre-zeroed); host keeps core 0's.
    out = nc.dram_tensor(
        "out", [E_PER_CORE + 1, 2, ROW], mybir.dt.float16, kind="ExternalOutput"
    )

    # bt viewed wrapped-16 (idx buffer layout) and partition-major-128 (scales)
    bt_w16 = bt.rearrange("(s p) -> p s", p=16)     # [16, 32]: bt[s*16+p]
    bt_p128 = bt.rearrange("(n p) -> p n", p=128)   # [128, 4]: bt[n*128+p]

    with (
        ExitStack() as stack,
        nc.Block() as block,
    ):
        bufs = [
            stack.enter_context(
                nc.sbuf_tensor(f"buf{i}", [128, slots, ROW], mybir.dt.float16)
            )
            for i in range(n_buf)
        ]
        bt32 = stack.enter_context(nc.sbuf_tensor("bt32", [128, 32], mybir.dt.int32))
        btp32 = stack.enter_context(nc.sbuf_tensor("btp32", [128, 4], mybir.dt.int32))
        btf = stack.enter_context(nc.sbuf_tensor("btf", [128, 32], mybir.dt.float32))
        btpf = stack.enter_context(nc.sbuf_tensor("btpf", [128, 4], mybir.dt.float32))
        valid = stack.enter_context(nc.sbuf_tensor("valid", [128, 4], mybir.dt.float32))
        k16 = stack.enter_context(nc.sbuf_tensor("k16", [128, 32], mybir.dt.int16))
        v16 = stack.enter_context(nc.sbuf_tensor("v16", [128, 32], mybir.dt.int16))
        ksv = stack.enter_context(nc.sbuf_tensor("ksv", [128, 4], mybir.dt.float32))
        vsv = stack.enter_context(nc.sbuf_tensor("vsv", [128, 4], mybir.dt.float32))
        scl = stack.enter_context(nc.sbuf_tensor("scl", [128, 2], mybir.dt.float32))

        load_sem = stack.enter_context(nc.semaphore("load"))
        vchain = stack.enter_context(nc.semaphore("vchain"))
        scale_sem = stack.enter_context(nc.semaphore("scale"))
        # Per-buffer DMA sems: concurrent DMAs on one shared sem would make
        # intermediate values ambiguous (increments from different DMAs mix).
        gather_sems = [
            stack.enter_context(nc.semaphore(f"gather{i}")) for i in range(n_buf)
        ]
        store_sems = [
            stack.enter_context(nc.semaphore(f"store{i}")) for i in range(n_buf)
        ]

        def gather_aps(gg):
            half, n = divmod(gg % n_gather, g_per_half)  # which half, which chunk
            idx = (k16 if half == 0 else v16)[:, cols * n : cols * (n + 1)]
            return half, n, idx

        def unit_aps(u):
            gg, s = divmod(u, slots)  # gather index, slot within buffer
            half, n, _ = gather_aps(gg)
            m = n * slots + s
            sc = (ksv if half == 0 else vsv)[:, m : m + 1]
            e0 = chunk * n + 128 * s
            dst = out[1 + e0 : 1 + e0 + 128, half, :]
            return gg, s, sc, dst

        n_gather_total = n_gather * n_reps
        n_units = n_gather_total * slots
        store_counts = [
            16 * slots * len([g for g in range(n_gather_total) if g % n_buf == b])
            for b in range(n_buf)
        ]

        def store_stream(eng, parity, n_engines):
            for u in range(n_units):
                if u % n_engines != parity:
                    continue
                gg, s, _, dst = unit_aps(u)
                eng.wait_ge(scale_sem, u + 1)
                eng.dma_start(dst, bufs[gg % n_buf][:, s, :]).then_inc(
                    store_sems[gg % n_buf], 16
                )

        @block.sync
        def _(sync: bass.BassEngine):
            # Prolog loads: bt wrapped-16 replicated into all 8 partition
            # groups, bt partition-major, and the scale pair.
            with nc.allow_non_contiguous_dma(reason="2KB one-time index loads"):
                for g in range(8):
                    sync.dma_start(
                        bt32[16 * g : 16 * g + 16, :], bt_w16[:, :]
                    ).then_inc(load_sem, 16)
                sync.dma_start(btp32[:, :], bt_p128[:, :]).then_inc(load_sem, 16)
            sync.dma_start(scl[:, :], scales[:, :]).then_inc(load_sem, 16)
            # Store loop
            store_stream(sync, 0, 2 if dual_store else 1)
            for b in range(n_buf):
                sync.wait_ge(store_sems[b], store_counts[b])

        if dual_store:

            @block.scalar
            def _(scalar: bass.BassEngine):
                store_stream(scalar, 1, 2)

        @block.vector
        def _(vector: bass.BassVectorEngine):
            vector.wait_ge(load_sem, 16 * 10)
            # Row indices: k = 2*bt, v = 2*bt + 1 (as int16, clamped >= 0).
            # Same-engine RAW chains need explicit sync (deep pipeline).
            vector.tensor_copy(btf[:, :], bt32[:, :]).then_inc(vchain, 1)
            vector.wait_ge(vchain, 1)
            vector.tensor_scalar_max(btf[:, :], btf[:, :], 0.0).then_inc(vchain, 1)
            vector.wait_ge(vchain, 2)
            vector.tensor_scalar_mul(k16[:, :], btf[:, :], 2.0).then_inc(vchain, 1)
            vector.tensor_scalar(
                v16[:, :], btf[:, :], 2.0, 1.0,
                op0=mybir.AluOpType.mult, op1=mybir.AluOpType.add,
            ).then_inc(vchain, 1)
            # Per-entry scales: (bt > 0) * {k,v}_scale, partition-major
            vector.tensor_copy(btpf[:, :], btp32[:, :]).then_inc(vchain, 1)
            vector.wait_ge(vchain, 5)
            vector.tensor_scalar(
                valid[:, :], btpf[:, :], 0.0, None, op0=mybir.AluOpType.is_gt
            ).then_inc(vchain, 1)
            vector.wait_ge(vchain, 6)
            vector.tensor_scalar_mul(ksv[:, :], valid[:, :], scl[:, 0:1]).then_inc(
                vchain, 1
            )
            vector.tensor_scalar_mul(vsv[:, :], valid[:, :], scl[:, 1:2]).then_inc(
                vchain, 1
            )
            vector.wait_ge(vchain, 8)
            # Dequant loop
            for u in range(n_units):
                gg, s, sc, _ = unit_aps(u)
                buf = bufs[gg % n_buf]
                vector.wait_ge(gather_sems[gg % n_buf], 16 * (gg // n_buf + 1))
                vector.tensor_scalar_mul(
                    buf[:, s : s + 1, :], buf[:, s : s + 1, :], sc
                ).then_inc(scale_sem, 1)

        @block.gpsimd
        def _(gpsimd: bass.BassGpSimd):
            gpsimd.load_library(mlp)
            gpsimd.wait_ge(vchain, 4)  # k16/v16 written
            for gg in range(n_gather_total):
                _, _, idx = gather_aps(gg)
                if gg >= n_buf:
                    gpsimd.wait_ge(store_sems[gg % n_buf], 16 * slots * (gg // n_buf))
                gpsimd.dma_gather(
                    bufs[gg % n_buf][:, :, :],
                    kv[:, :],
                    idx,
                    chunk,
                    chunk,
                    ROW,
                    single_packet=single_packet,
                    # sems are queue-locked: keep queue a function of buffer
                    queue_num=(gg % n_buf) % n_queues,
                ).then_inc(gather_sems[gg % n_buf], 16)

    nc.compile()
    return nc


def _get_nc() -> bass.Bass:
    global _NC_CACHE
    if _NC_CACHE is None:
        _NC_CACHE = build_nc()
    return _NC_CACHE


def _make_in_maps(inputs):
    kv = np.ascontiguousarray(np.asarray(inputs["kv_cache"])).view(np.float16)
    bt = np.asarray(inputs["block_tables"])
    k_scale = np.float32(inputs["k_scale"])
    v_scale = np.float32(inputs["v_scale"])

    kv_flat = kv.reshape(N_ROWS, ROW)
    scales = np.empty((128, 2), np.float32)
    scales[:, 0] = k_scale
    scales[:, 1] = v_scale

    seq_per_core = BATCH // N_CORES
    in_maps = []
    for c in range(N_CORES):
        bt_shard = np.ascontiguousarray(
            bt[seq_per_core * c : seq_per_core * (c + 1)]
            .reshape(-1)
            .astype(np.int32)
        )
        in_maps.append({"kv": kv_flat, "bt": bt_shard, "scales": scales})
    return in_maps


def _run(inputs, **kwargs) -> tuple[np.ndarray, "bass_utils.BassKernelResults"]:
    res = bass_utils.run_bass_kernel_spmd(
        _get_nc(), _make_in_maps(inputs), core_ids=list(range(N_CORES)), **kwargs
    )
    outs = [r["out"] for r in res.results]  # each [513, 2, ROW] fp16
    full = np.empty((BATCH * MAX_BLOCKS_PER_SEQ + 1, 2, ROW), np.float16)
    full[0] = outs[0][0]
    for c in range(N_CORES):
        full[1 + E_PER_CORE * c : 1 + E_PER_CORE * (c + 1)] = outs[c][1:]
    return (
        full.reshape(-1, 2, NUM_KV_HEADS, BLOCK_SIZE, HEAD_DIM),
        res,
    )


def kernel(**inputs) -> np.ndarray:
    out, _ = _run(inputs)
    return out



# revision 2
# speedup vs baseline: 519.6975x; 519.6975x over previous
"""Paged KV-cache gather + dequant kernel for 8 Trainium2 NeuronCores (v3).

Problem: out[0] = zeros; out[1+i] = kv_cache[block_tables.flat[i]] * scale
(k_scale for the K half, v_scale for the V half), zeroed where the table
entry is <= 0.  Shapes: kv_cache [4096, 2, 8, 16, 128] fp16,
block_tables [32, 128] int, out [4097, 2, 8, 16, 128] fp16.

Sharding: batch across the 8 cores (4 sequences = 512 entries per core);
kv_cache replicated.  v3 is HWDGE-only — no GPSIMD, hence no Q7 library
reload (which dominates the one-shot NEFF time of the SWDGE design):
  - the host passes per-entry element offsets (max(bt,0) * 32768) and
    per-entry scales ((bt > 0) * k/v_scale, partition-major),
  - each entry is gathered with a dynamic-offset HWDGE dma_start (64KB
    contiguous into one SBUF partition); entries alternate between the
    sync and scalar engine queues so both HWDGE rings stream,
  - DVE multiplies each 128-entry chunk by its per-partition scales,
  - each chunk is stored with one fully-contiguous 8MB DMA.
"""

import sys

if "/opt/trn_rl_repo" not in sys.path:
    sys.path.insert(0, "/opt/trn_rl_repo")

from contextlib import ExitStack

import numpy as np

import concourse.bacc as bacc
import concourse.bass as bass
import concourse.mybir as mybir
from concourse import bass_utils
from concourse._compat import get_trn_type

N_CORES = 8
NUM_BLOCKS, NUM_KV_HEADS, HEAD_DIM, BLOCK_SIZE = 4096, 8, 128, 16
BATCH, MAX_BLOCKS_PER_SEQ = 32, 128

ROW = NUM_KV_HEADS * BLOCK_SIZE * HEAD_DIM  # 16384 fp16 = one K or V half-block
N_ROWS = NUM_BLOCKS * 2                     # 8192
ENTRY = 2 * ROW                             # 32768 fp16 = one full 64KB entry
E_PER_CORE = (BATCH // N_CORES) * MAX_BLOCKS_PER_SEQ  # 512 entries per core
N_CHUNKS = E_PER_CORE // 128                # 4 chunks of 128 entries per rep
N_BUF = 3
N_REGS = 8

_NC_CACHE = None


def build_nc(n_reps: int = 1) -> bass.Bass:
    nc = bacc.Bacc(get_trn_type() or "TRN2")

    kv = nc.dram_tensor("kv", [N_ROWS * ROW], mybir.dt.float16, kind="ExternalInput")
    offs = nc.dram_tensor("offs", [E_PER_CORE], mybir.dt.int32, kind="ExternalInput")
    scp_d = nc.dram_tensor(
        "scp", [128, 2 * N_CHUNKS], mybir.dt.float32, kind="ExternalInput"
    )
    # out block 0 stays zero (buffers are pre-zeroed); host keeps core 0's.
    out = nc.dram_tensor(
        "out", [E_PER_CORE + 1, 2, ROW], mybir.dt.float16, kind="ExternalOutput"
    )

    n_total = N_CHUNKS * n_reps

    with (
        ExitStack() as stack,
        nc.Block() as block,
    ):
        bufs = [
            stack.enter_context(
                nc.sbuf_tensor(f"buf{i}", [128, 2, ROW], mybir.dt.float16)
            )
            for i in range(N_BUF)
        ]
        offs_sb = stack.enter_context(
            nc.sbuf_tensor("offs_sb", [1, E_PER_CORE], mybir.dt.int32)
        )
        scp = stack.enter_context(
            nc.sbuf_tensor("scps", [128, 2 * N_CHUNKS], mybir.dt.float32)
        )

        load_sem = stack.enter_context(nc.semaphore("load"))
        scale_sem = stack.enter_context(nc.semaphore("scale"))
        gA = [stack.enter_context(nc.semaphore(f"gA{i}")) for i in range(N_BUF)]
        gB = [stack.enter_context(nc.semaphore(f"gB{i}")) for i in range(N_BUF)]
        ssem = [stack.enter_context(nc.semaphore(f"ss{i}")) for i in range(N_BUF)]

        def gather_stream(eng, regs, parity, gsems):
            # 64 entries of each 128-entry chunk per engine (even/odd split)
            eng.wait_ge(load_sem, 32)
            k = 0
            for cc in range(n_total):
                c = cc % N_CHUNKS
                b = cc % N_BUF
                if cc >= N_BUF:
                    eng.wait_ge(ssem[b], 16 * (cc // N_BUF))
                for p in range(parity, 128, 2):
                    e = 128 * c + p
                    r = regs[k % N_REGS]
                    k += 1
                    eng.reg_load(r, offs_sb[0:1, e : e + 1])
                    src = bass.AP(kv[:].tensor, r, [[1, ENTRY]])
                    eng.dma_start(bufs[b][p : p + 1, :, :], src).then_inc(
                        gsems[b], 16
                    )

        @block.sync
        def _(sync: bass.BassEngine):
            sync.dma_start(offs_sb[0:1, :], offs[:].unsqueeze(0)).then_inc(
                load_sem, 16
            )
            sync.dma_start(scp[:, :], scp_d[:, :]).then_inc(load_sem, 16)
            regs = [
                nc.alloc_register(mybir.EngineType.SP, f"ra{i}") for i in range(N_REGS)
            ]
            gather_stream(sync, regs, 0, gA)

        @block.scalar
        def _(scalar: bass.BassEngine):
            regs = [
                nc.alloc_register(mybir.EngineType.Activation, f"rb{i}")
                for i in range(N_REGS)
            ]
            # odd entries, with the chunk store trailing two chunks behind so
            # the store's scale_sem wait never blocks steady-state issue
            scalar.wait_ge(load_sem, 32)
            k = 0

            def store(cc):
                b = cc % N_BUF
                e0 = 128 * (cc % N_CHUNKS)
                scalar.wait_ge(scale_sem, 2 * (cc + 1))
                scalar.dma_start(
                    out[1 + e0 : 1 + e0 + 128, :, :], bufs[b][:, :, :]
                ).then_inc(ssem[b], 16)

            for cc in range(n_total):
                c = cc % N_CHUNKS
                b = cc % N_BUF
                if cc >= N_BUF:
                    scalar.wait_ge(ssem[b], 16 * (cc // N_BUF))
                for p in range(1, 128, 2):
                    e = 128 * c + p
                    r = regs[k % N_REGS]
                    k += 1
                    scalar.reg_load(r, offs_sb[0:1, e : e + 1])
                    src = bass.AP(kv[:].tensor, r, [[1, ENTRY]])
                    scalar.dma_start(bufs[b][p : p + 1, :, :], src).then_inc(
                        gB[b], 16
                    )
                if cc >= 2:
                    store(cc - 2)
            for cc in range(max(n_total - 2, 0), n_total):
                store(cc)

        @block.vector
        def _(vector: bass.BassVectorEngine):
            for cc in range(n_total):
                c = cc % N_CHUNKS
                b = cc % N_BUF
                buf = bufs[b]
                vector.wait_ge(gA[b], 1024 * (cc // N_BUF + 1))
                vector.wait_ge(gB[b], 1024 * (cc // N_BUF + 1))
                vector.tensor_scalar_mul(
                    buf[:, 0:1, :], buf[:, 0:1, :], scp[:, c : c + 1]
                ).then_inc(scale_sem, 1)
                vector.tensor_scalar_mul(
                    buf[:, 1:2, :], buf[:, 1:2, :],
                    scp[:, N_CHUNKS + c : N_CHUNKS + c + 1],
                ).then_inc(scale_sem, 1)

    nc.compile()
    return nc


def _get_nc() -> bass.Bass:
    global _NC_CACHE
    if _NC_CACHE is None:
        _NC_CACHE = build_nc()
    return _NC_CACHE


def _make_in_maps(inputs):
    kv = np.ascontiguousarray(np.asarray(inputs["kv_cache"])).view(np.float16)
    bt = np.asarray(inputs["block_tables"]).astype(np.int64)
    k_scale = np.float32(inputs["k_scale"])
    v_scale = np.float32(inputs["v_scale"])

    kv_flat = kv.reshape(N_ROWS * ROW)

    seq_per_core = BATCH // N_CORES
    in_maps = []
    for c in range(N_CORES):
        bt_shard = bt[seq_per_core * c : seq_per_core * (c + 1)].reshape(-1)
        offs = (np.clip(bt_shard, 0, None) * ENTRY).astype(np.int32)
        valid = bt_shard > 0
        # per-entry scales, partition-major: entry e = c*128 + p -> [p, c]
        vm = valid.reshape(N_CHUNKS, 128).T.astype(np.float32)  # [128, N_CHUNKS]
        scp = np.concatenate([vm * k_scale, vm * v_scale], axis=1)  # [128, 8]
        in_maps.append(
            {
                "kv": kv_flat,
                "offs": np.ascontiguousarray(offs),
                "scp": np.ascontiguousarray(scp),
            }
        )
    return in_maps


def _run(inputs, **kwargs):
    res = bass_utils.run_bass_kernel_spmd(
        _get_nc(), _make_in_maps(inputs), core_ids=list(range(N_CORES)), **kwargs
    )
    outs = [r["out"] for r in res.results]  # each [513, 2, ROW] fp16
    full = np.empty((BATCH * MAX_BLOCKS_PER_SEQ + 1, 2, ROW), np.float16)
    full[0] = outs[0][0]
    for c in range(N_CORES):
        full[1 + E_PER_CORE * c : 1 + E_PER_CORE * (c + 1)] = outs[c][1:]
    return (
        full.reshape(-1, 2, NUM_KV_HEADS, BLOCK_SIZE, HEAD_DIM),
        res,
    )


def kernel(**inputs) -> np.ndarray:
    out, _ = _run(inputs)
    return out


# revision 5
# speedup vs baseline: 524.4030x; 1.0091x over previous
"""Paged KV-cache gather + dequant kernel for 8 Trainium2 NeuronCores (v4).

Problem: out[0] = zeros; out[1+i] = kv_cache[block_tables.flat[i]] * scale
(k_scale for the K half, v_scale for the V half), zeroed where the table
entry is <= 0.  Shapes: kv_cache [4096, 2, 8, 16, 128] fp16,
block_tables [32, 128] int, out [4097, 2, 8, 16, 128] fp16.

Sharding: batch across the 8 cores (4 sequences = 512 entries per core);
kv_cache replicated.  v4 is HWDGE-only — no GPSIMD, hence no Q7 library
reload (which dominates the one-shot NEFF time of the SWDGE design):
  - the host passes per-entry element offsets (max(bt,0) * 32768) and
    per-entry scales ((bt > 0) * k/v_scale, partition-major),
  - each entry is gathered with a dynamic-offset HWDGE dma_start (64KB
    contiguous into one SBUF partition, single-packet); entries alternate
    between the sync and scalar engine queues, issued in a stride-4
    partition order so consecutive DMAs land on different SDMA engines,
  - DVE multiplies each 128-entry chunk by its per-partition scales,
  - each chunk is stored with one fully-contiguous 8MB DMA.
"""

import sys

if "/opt/trn_rl_repo" not in sys.path:
    sys.path.insert(0, "/opt/trn_rl_repo")

from contextlib import ExitStack

import numpy as np

import concourse.bacc as bacc
import concourse.bass as bass
import concourse.mybir as mybir
from concourse import bass_utils
from concourse._compat import get_trn_type

N_CORES = 8
NUM_BLOCKS, NUM_KV_HEADS, HEAD_DIM, BLOCK_SIZE = 4096, 8, 128, 16
BATCH, MAX_BLOCKS_PER_SEQ = 32, 128

ROW = NUM_KV_HEADS * BLOCK_SIZE * HEAD_DIM  # 16384 fp16 = one K or V half-block
N_ROWS = NUM_BLOCKS * 2                     # 8192
ENTRY = 2 * ROW                             # 32768 fp16 = one full 64KB entry
E_PER_CORE = (BATCH // N_CORES) * MAX_BLOCKS_PER_SEQ  # 512 entries per core
N_CHUNKS = E_PER_CORE // 128                # 4 chunks of 128 entries per rep
N_BUF = 3
N_REGS = 8

_NC_CACHE = None


def build_nc(n_reps: int = 1) -> bass.Bass:
    nc = bacc.Bacc(get_trn_type() or "TRN2")

    kv = nc.dram_tensor("kv", [N_ROWS * ROW], mybir.dt.float16, kind="ExternalInput")
    offs = nc.dram_tensor("offs", [E_PER_CORE], mybir.dt.int32, kind="ExternalInput")
    scp_d = nc.dram_tensor(
        "scp", [128, 2 * N_CHUNKS], mybir.dt.float32, kind="ExternalInput"
    )
    # out block 0 stays zero (buffers are pre-zeroed); host keeps core 0's.
    out = nc.dram_tensor(
        "out", [E_PER_CORE + 1, 2, ROW], mybir.dt.float16, kind="ExternalOutput"
    )

    n_total = N_CHUNKS * n_reps

    with (
        ExitStack() as stack,
        nc.Block() as block,
    ):
        bufs = [
            stack.enter_context(
                nc.sbuf_tensor(f"buf{i}", [128, 2, ROW], mybir.dt.float16)
            )
            for i in range(N_BUF)
        ]
        offs_sb = stack.enter_context(
            nc.sbuf_tensor("offs_sb", [1, E_PER_CORE], mybir.dt.int32)
        )
        scp = stack.enter_context(
            nc.sbuf_tensor("scps", [128, 2 * N_CHUNKS], mybir.dt.float32)
        )

        load_sem = stack.enter_context(nc.semaphore("load"))
        scale_sem = stack.enter_context(nc.semaphore("scale"))
        gA = [stack.enter_context(nc.semaphore(f"gA{i}")) for i in range(N_BUF)]
        gB = [stack.enter_context(nc.semaphore(f"gB{i}")) for i in range(N_BUF)]
        ssem = [stack.enter_context(nc.semaphore(f"ss{i}")) for i in range(N_BUF)]

        def gather_stream(eng, regs, parity, gsems):
            # 64 entries of each 128-entry chunk per engine (even/odd split);
            # partitions issued in a stride-4 order so consecutive DMAs land
            # on different SDMA engines, and single-packet descriptors.
            eng.wait_ge(load_sem, 32)
            k = 0
            p_order = [p for s in range(parity, 8, 2) for p in range(s, 128, 8)]
            for cc in range(n_total):
                c = cc % N_CHUNKS
                b = cc % N_BUF
                if cc >= N_BUF:
                    eng.wait_ge(ssem[b], 16 * (cc // N_BUF))
                for p in p_order:
                    e = 128 * c + p
                    r = regs[k % N_REGS]
                    k += 1
                    eng.reg_load(r, offs_sb[0:1, e : e + 1])
                    src = bass.AP(kv[:].tensor, r, [[1, ENTRY]])
                    eng.dma_start(
                        bufs[b][p : p + 1, :, :], src, single_packet=True
                    ).then_inc(gsems[b], 16)

        @block.sync
        def _(sync: bass.BassEngine):
            sync.dma_start(offs_sb[0:1, :], offs[:].unsqueeze(0)).then_inc(
                load_sem, 16
            )
            sync.dma_start(scp[:, :], scp_d[:, :]).then_inc(load_sem, 16)
            regs = [
                nc.alloc_register(mybir.EngineType.SP, f"ra{i}") for i in range(N_REGS)
            ]
            gather_stream(sync, regs, 0, gA)

        @block.scalar
        def _(scalar: bass.BassEngine):
            regs = [
                nc.alloc_register(mybir.EngineType.Activation, f"rb{i}")
                for i in range(N_REGS)
            ]
            # odd entries, with the chunk store trailing two chunks behind so
            # the store's scale_sem wait never blocks steady-state issue
            scalar.wait_ge(load_sem, 32)
            k = 0

            def store(cc):
                b = cc % N_BUF
                e0 = 128 * (cc % N_CHUNKS)
                scalar.wait_ge(scale_sem, 2 * (cc + 1))
                scalar.dma_start(
                    out[1 + e0 : 1 + e0 + 128, :, :], bufs[b][:, :, :]
                ).then_inc(ssem[b], 16)

            for cc in range(n_total):
                c = cc % N_CHUNKS
                b = cc % N_BUF
                if cc >= N_BUF:
                    scalar.wait_ge(ssem[b], 16 * (cc // N_BUF))
                for p in range(1, 128, 2):
                    e = 128 * c + p
                    r = regs[k % N_REGS]
                    k += 1
                    scalar.reg_load(r, offs_sb[0:1, e : e + 1])
                    src = bass.AP(kv[:].tensor, r, [[1, ENTRY]])
                    scalar.dma_start(bufs[b][p : p + 1, :, :], src).then_inc(
                        gB[b], 16
                    )
                if cc >= 2:
                    store(cc - 2)
            for cc in range(max(n_total - 2, 0), n_total):
                store(cc)

        @block.vector
        def _(vector: bass.BassVectorEngine):
            for cc in range(n_total):
                c = cc % N_CHUNKS
                b = cc % N_BUF
                buf = bufs[b]
                vector.wait_ge(gA[b], 1024 * (cc // N_BUF + 1))
                vector.wait_ge(gB[b], 1024 * (cc // N_BUF + 1))
                vector.tensor_scalar_mul(
                    buf[:, 0:1, :], buf[:, 0:1, :], scp[:, c : c + 1]
                ).then_inc(scale_sem, 1)
                vector.tensor_scalar_mul(
                    buf[:, 1:2, :], buf[:, 1:2, :],
                    scp[:, N_CHUNKS + c : N_CHUNKS + c + 1],
                ).then_inc(scale_sem, 1)

    nc.compile()
    return nc


def _get_nc() -> bass.Bass:
    global _NC_CACHE
    if _NC_CACHE is None:
        _NC_CACHE = build_nc()
    return _NC_CACHE


def _make_in_maps(inputs):
    kv = np.ascontiguousarray(np.asarray(inputs["kv_cache"])).view(np.float16)
    bt = np.asarray(inputs["block_tables"]).astype(np.int64)
    k_scale = np.float32(inputs["k_scale"])
    v_scale = np.float32(inputs["v_scale"])

    kv_flat = kv.reshape(N_ROWS * ROW)

    seq_per_core = BATCH // N_CORES
    in_maps = []
    for c in range(N_CORES):
        bt_shard = bt[seq_per_core * c : seq_per_core * (c + 1)].reshape(-1)
        offs = (np.clip(bt_shard, 0, None) * ENTRY).astype(np.int32)
        valid = bt_shard > 0
        # per-entry scales, partition-major: entry e = c*128 + p -> [p, c]
        vm = valid.reshape(N_CHUNKS, 128).T.astype(np.float32)  # [128, N_CHUNKS]
        scp = np.concatenate([vm * k_scale, vm * v_scale], axis=1)  # [128, 8]
        in_maps.append(
            {
                "kv": kv_flat,
                "offs": np.ascontiguousarray(offs),
                "scp": np.ascontiguousarray(scp),
            }
        )
    return in_maps


def _run(inputs, **kwargs):
    res = bass_utils.run_bass_kernel_spmd(
        _get_nc(), _make_in_maps(inputs), core_ids=list(range(N_CORES)), **kwargs
    )
    outs = [r["out"] for r in res.results]  # each [513, 2, ROW] fp16
    full = np.empty((BATCH * MAX_BLOCKS_PER_SEQ + 1, 2, ROW), np.float16)
    full[0] = outs[0][0]
    for c in range(N_CORES):
        full[1 + E_PER_CORE * c : 1 + E_PER_CORE * (c + 1)] = outs[c][1:]
    return (
        full.reshape(-1, 2, NUM_KV_HEADS, BLOCK_SIZE, HEAD_DIM),
        res,
    )


def kernel(**inputs) -> np.ndarray:
    out, _ = _run(inputs)
    return out


# revision 6
# speedup vs baseline: 983.2328x; 1.8750x over previous
"""Paged KV-cache gather + dequant kernel for 8 Trainium2 NeuronCores (v6).

Problem: out[0] = zeros; out[1+i] = kv_cache[block_tables.flat[i]] * scale
(k_scale for the K half, v_scale for the V half), zeroed where the table
entry is <= 0.  Shapes: kv_cache [4096, 2, 8, 16, 128] fp16,
block_tables [32, 128] int, out [4097, 2, 8, 16, 128] fp16.

Sharding: batch across the 8 cores (4 sequences = 512 entries per core);
kv_cache replicated.  v4 is HWDGE-only — no GPSIMD, hence no Q7 library
reload (which dominates the one-shot NEFF time of the SWDGE design):
  - the host passes per-entry element offsets (max(bt,0) * 32768) and
    per-entry scales ((bt > 0) * k/v_scale, partition-major),
  - each entry is gathered with a dynamic-offset HWDGE dma_start (64KB
    contiguous into one SBUF partition, single-packet); entries alternate
    between the sync and scalar engine queues, issued in a stride-4
    partition order so consecutive DMAs land on different SDMA engines,
  - DVE multiplies each 128-entry chunk by its per-partition scales,
  - each chunk is stored with one fully-contiguous 8MB DMA.
"""

import sys

if "/opt/trn_rl_repo" not in sys.path:
    sys.path.insert(0, "/opt/trn_rl_repo")

from contextlib import ExitStack

import numpy as np

import concourse.bacc as bacc
import concourse.bass as bass
import concourse.mybir as mybir
from concourse import bass_utils
from concourse._compat import get_trn_type

N_CORES = 8
NUM_BLOCKS, NUM_KV_HEADS, HEAD_DIM, BLOCK_SIZE = 4096, 8, 128, 16
BATCH, MAX_BLOCKS_PER_SEQ = 32, 128

ROW = NUM_KV_HEADS * BLOCK_SIZE * HEAD_DIM  # 16384 fp16 = one K or V half-block
N_ROWS = NUM_BLOCKS * 2                     # 8192
ENTRY = 2 * ROW                             # 32768 fp16 = one full 64KB entry
E_PER_CORE = (BATCH // N_CORES) * MAX_BLOCKS_PER_SEQ  # 512 entries per core
N_CHUNKS = E_PER_CORE // 128                # 4 chunks of 128 entries per rep
N_BUF = 3
N_REGS = 8

_NC_CACHE = None


def build_nc(n_reps: int = 1) -> bass.Bass:
    nc = bacc.Bacc(get_trn_type() or "TRN2")

    kv = nc.dram_tensor("kv", [N_ROWS * ROW], mybir.dt.float16, kind="ExternalInput")
    offs = nc.dram_tensor("offs", [E_PER_CORE], mybir.dt.int32, kind="ExternalInput")
    scp_d = nc.dram_tensor(
        "scp", [128, 2 * N_CHUNKS], mybir.dt.float32, kind="ExternalInput"
    )
    # out block 0 stays zero (buffers are pre-zeroed); host keeps core 0's.
    out = nc.dram_tensor(
        "out", [E_PER_CORE + 1, 2, ROW], mybir.dt.float16, kind="ExternalOutput"
    )

    n_total = N_CHUNKS * n_reps

    with (
        ExitStack() as stack,
        nc.Block() as block,
    ):
        bufs = [
            stack.enter_context(
                nc.sbuf_tensor(f"buf{i}", [128, 2, ROW], mybir.dt.float16)
            )
            for i in range(N_BUF)
        ]
        offs_sb = stack.enter_context(
            nc.sbuf_tensor("offs_sb", [1, E_PER_CORE], mybir.dt.int32)
        )
        scp = stack.enter_context(
            nc.sbuf_tensor("scps", [128, 2 * N_CHUNKS], mybir.dt.float32)
        )

        load_sem = stack.enter_context(nc.semaphore("load"))
        scale_sem = stack.enter_context(nc.semaphore("scale"))
        gA = [stack.enter_context(nc.semaphore(f"gA{i}")) for i in range(N_BUF)]
        gB = [stack.enter_context(nc.semaphore(f"gB{i}")) for i in range(N_BUF)]
        ssem = [stack.enter_context(nc.semaphore(f"ss{i}")) for i in range(N_BUF)]

        def gather_stream(eng, regs, parity, gsems):
            # 64 entries of each 128-entry chunk per engine (even/odd split);
            # partitions issued in a stride-4 order so consecutive DMAs land
            # on different SDMA engines, and single-packet descriptors.
            eng.wait_ge(load_sem, 32)
            k = 0
            p_order = [p for s in range(parity, 8, 2) for p in range(s, 128, 8)]
            for cc in range(n_total):
                c = cc % N_CHUNKS
                b = cc % N_BUF
                if cc >= N_BUF:
                    eng.wait_ge(ssem[b], 16 * (cc // N_BUF))
                for p in p_order:
                    e = 128 * c + p
                    r = regs[k % N_REGS]
                    k += 1
                    eng.reg_load(r, offs_sb[0:1, e : e + 1])
                    src = bass.AP(kv[:].tensor, r, [[1, ENTRY]])
                    eng.dma_start(
                        bufs[b][p : p + 1, :, :], src, single_packet=True
                    ).then_inc(gsems[b], 16)

        @block.sync
        def _(sync: bass.BassEngine):
            sync.dma_start(offs_sb[0:1, :], offs[:].unsqueeze(0)).then_inc(
                load_sem, 16
            )
            sync.dma_start(scp[:, :], scp_d[:, :]).then_inc(load_sem, 16)
            regs = [
                nc.alloc_register(mybir.EngineType.SP, f"ra{i}") for i in range(N_REGS)
            ]
            gather_stream(sync, regs, 0, gA)

        @block.scalar
        def _(scalar: bass.BassEngine):
            regs = [
                nc.alloc_register(mybir.EngineType.Activation, f"rb{i}")
                for i in range(N_REGS)
            ]
            # odd entries, with the chunk store trailing two chunks behind so
            # the store's scale_sem wait never blocks steady-state issue
            scalar.wait_ge(load_sem, 32)
            k = 0
            p_order = [p for s in range(1, 8, 2) for p in range(s, 128, 8)]

            def store(cc):
                b = cc % N_BUF
                e0 = 128 * (cc % N_CHUNKS)
                scalar.wait_ge(scale_sem, 2 * (cc + 1))
                scalar.dma_start(
                    out[1 + e0 : 1 + e0 + 128, :, :], bufs[b][:, :, :]
                ).then_inc(ssem[b], 16)

            for cc in range(n_total):
                c = cc % N_CHUNKS
                b = cc % N_BUF
                if cc >= N_BUF:
                    scalar.wait_ge(ssem[b], 16 * (cc // N_BUF))
                for p in p_order:
                    e = 128 * c + p
                    r = regs[k % N_REGS]
                    k += 1
                    scalar.reg_load(r, offs_sb[0:1, e : e + 1])
                    src = bass.AP(kv[:].tensor, r, [[1, ENTRY]])
                    scalar.dma_start(
                        bufs[b][p : p + 1, :, :], src, single_packet=True
                    ).then_inc(gB[b], 16)
                if cc >= 2:
                    store(cc - 2)
            for cc in range(max(n_total - 2, 0), n_total):
                store(cc)

        @block.vector
        def _(vector: bass.BassVectorEngine):
            for cc in range(n_total):
                c = cc % N_CHUNKS
                b = cc % N_BUF
                buf = bufs[b]
                vector.wait_ge(gA[b], 1024 * (cc // N_BUF + 1))
                vector.wait_ge(gB[b], 1024 * (cc // N_BUF + 1))
                vector.tensor_scalar_mul(
                    buf[:, 0:1, :], buf[:, 0:1, :], scp[:, c : c + 1]
                ).then_inc(scale_sem, 1)
                vector.tensor_scalar_mul(
                    buf[:, 1:2, :], buf[:, 1:2, :],
                    scp[:, N_CHUNKS + c : N_CHUNKS + c + 1],
                ).then_inc(scale_sem, 1)

    nc.compile()
    return nc


def _get_nc() -> bass.Bass:
    global _NC_CACHE
    if _NC_CACHE is None:
        _NC_CACHE = build_nc()
    return _NC_CACHE


def _make_in_maps(inputs):
    kv = np.ascontiguousarray(np.asarray(inputs["kv_cache"])).view(np.float16)
    bt = np.asarray(inputs["block_tables"]).astype(np.int64)
    k_scale = np.float32(inputs["k_scale"])
    v_scale = np.float32(inputs["v_scale"])

    kv_flat = kv.reshape(N_ROWS * ROW)

    seq_per_core = BATCH // N_CORES
    in_maps = []
    for c in range(N_CORES):
        bt_shard = bt[seq_per_core * c : seq_per_core * (c + 1)].reshape(-1)
        offs = (np.clip(bt_shard, 0, None) * ENTRY).astype(np.int32)
        valid = bt_shard > 0
        # per-entry scales, partition-major: entry e = c*128 + p -> [p, c]
        vm = valid.reshape(N_CHUNKS, 128).T.astype(np.float32)  # [128, N_CHUNKS]
        scp = np.concatenate([vm * k_scale, vm * v_scale], axis=1)  # [128, 8]
        in_maps.append(
            {
                "kv": kv_flat,
                "offs": np.ascontiguousarray(offs),
                "scp": np.ascontiguousarray(scp),
            }
        )
    return in_maps


def _run(inputs, **kwargs):
    res = bass_utils.run_bass_kernel_spmd(
        _get_nc(), _make_in_maps(inputs), core_ids=list(range(N_CORES)), **kwargs
    )
    outs = [r["out"] for r in res.results]  # each [513, 2, ROW] fp16
    full = np.empty((BATCH * MAX_BLOCKS_PER_SEQ + 1, 2, ROW), np.float16)
    full[0] = outs[0][0]
    for c in range(N_CORES):
        full[1 + E_PER_CORE * c : 1 + E_PER_CORE * (c + 1)] = outs[c][1:]
    return (
        full.reshape(-1, 2, NUM_KV_HEADS, BLOCK_SIZE, HEAD_DIM),
        res,
    )


def kernel(**inputs) -> np.ndarray:
    out, _ = _run(inputs)
    return out
